# revision 67
# baseline (speedup 1.0000x reference)
"""TRN2 Bass kernel for 3-layer GAT + BN + MLP classifier (GATSBMs).

Self-contained: hardcodes all shapes. Accepts FULL inputs, returns FULL output.

Sharding: nodes sorted by in-degree desc -> rank r; core=r%8, slot j=r//8,
block b=j//128, lane p=j%128 (49 blocks of 128 lanes per core, NPAD=50176).
Each core owns its nodes' incoming edges. Per layer: each core computes
feat/el/er for its nodes (PE bf16 matmul vs W_ext=[W|W@albd|W@arbd], d-major
output channels), AllGather of 512B rows [feat bf16 | el f32 | pad] into a
per-core table, then dma_gathers over K=5 overlapping 32768-row int16
windows. Per-block window capacities come from a per-block mini-LP over the
interval-Hall feasibility constraints of the per-dst window-assignment flow;
a vectorized greedy then assigns each edge a (window, slot). The per-window
edge strips are contiguous across blocks, so gathers are emitted as
8-slot-aligned 1024-index chunks into per-window SBUF ring buffers
(chunk-packing the SWDGE 994ns fixed overhead); prefetch is two-phase per
block (own slots before the block's reads are emitted, deeper prefetch
after) so a ring-lap rewrite can never be ordered ahead of the previous
lap's readers. Edge softmax runs as DVE-add(el+er) -> Act Prelu(0.2) ->
Act Exp (bf16 out) -> DVE reduce; messages are scaled in place in the ring
(DVE bf16) and reduced on PE via identity-matmul PSUM accumulation. Phase B
and C are fused per block and software-pipelined at a 1-block lag; the
phase-A head (BN+ELU with exp(sc*x+bs) fused on Act, linear path on the
idle Pool engine) interleaves its two passes at a 4-block lag. BN stats via
PE ones-matmuls + AllGather + local sum. BN apply of layers 0/1 is fused
into the next layer's per-block head; layer 2's into the classifier chunks.
"""
import numpy as np

N = 50000
E = 850000
HD = 128
H = 4
D = 32
NCORES = 8
BLK = 128
NB = 49
NPC = NB * BLK          # 6272
NPAD = NCORES * NPC     # 50176
NEG = 0.2
BN_EPS = 1e-5
EL_DUMMY = -100.0
RW = 128                # f32 words per 512B table row: 64 feat + 4 el + 60 pad
WIN = 32768
K = 5                   # number of overlapping gather windows
WSS = tuple(int(round(i * (NPAD - WIN) / (K - 1))) for i in range(K))
# in-window dummy rows (el = -100): invalid node slots (block 48, lane >=106)
# exist at row k*NPC+6250 for k=0..7; pick one inside each window.
WDUM = tuple(next(k * NPC + 6250 - WSS[w] for k in range(NCORES)
                  if WSS[w] <= k * NPC + 6250 < WSS[w] + WIN)
             for w in range(K))
NMAX = 1024             # max indices per dma_gather (SWDGE ring safe)
_INTERP_DEBUG = False   # replace Prelu with DVE ops (interp lacks Prelu)

_CACHE = {}


def _cumcount(groups_sorted):
    """In-group running index for a group-sorted array."""
    n = len(groups_sorted)
    if n == 0:
        return np.zeros(0, np.int64)
    first = np.r_[True, groups_sorted[1:] != groups_sorted[:-1]]
    idx = np.arange(n, dtype=np.int64)
    start = idx[first][np.cumsum(first) - 1]
    return idx - start


def _build_plan(src, dst):
    """Vectorized host preprocessing of the graph structure."""
    src = np.asarray(src, dtype=np.int64)
    dst = np.asarray(dst, dtype=np.int64)
    deg = np.bincount(dst, minlength=N)
    order = np.argsort(-deg, kind="stable")
    rank_of = np.empty(N, dtype=np.int64)
    rank_of[order] = np.arange(N)

    srk = rank_of[src]
    row = NPC * (srk % NCORES) + srk // NCORES   # table row of the source

    # window membership per edge: windows form overlapping intervals, so each
    # row belongs to a contiguous window range [f, l].
    st = np.array(WSS)
    f = np.full(E, K, np.int64)
    l = np.full(E, -1, np.int64)
    for w in range(K - 1, -1, -1):
        inw = (row >= st[w]) & (row < st[w] + WIN)
        f[inw] = w
    for w in range(K):
        inw = (row >= st[w]) & (row < st[w] + WIN)
        l[inw] = w
    assert (f <= l).all()

    def blockmax(x):
        xr = np.zeros(NPAD, np.int64)
        xr[:N] = x[order]
        return xr[:NB * 1024].reshape(NB, 1024).max(axis=1)

    cats = [(i, j) for i in range(K) for j in range(i, K)
            if ((f == i) & (l == j)).any()]
    ncat = {(i, j): np.bincount(dst[(f == i) & (l == j)], minlength=N)
            for (i, j) in cats}

    # Hall interval constraints: for every window interval [i..j], the edges
    # confined to it must fit in its capacity.
    icons = []
    for i in range(K):
        for j in range(i, K):
            tot = np.zeros(N, np.int64)
            for (a, b2), c in ncat.items():
                if a >= i and b2 <= j:
                    tot += c
            icons.append(((i, j), blockmax(tot)))

    # per-block mini-LP (brute force over K-1 caps; last cap derived)
    CW = np.zeros((K, NB), np.int64)
    import itertools
    for b in range(NB):
        cons = [(ij, int(v[b])) for ij, v in icons]
        lb = [0] * K
        for (i, j), v in cons:
            if i == j:
                lb[i] = max(lb[i], v)
        best = None
        for combo in itertools.product(
                *[range(lb[w], lb[w] + 8) for w in range(K - 1)]):
            cK = lb[K - 1]
            ok = True
            for (i, j), v in cons:
                s = sum(combo[w] for w in range(i, min(j + 1, K - 1)))
                if j == K - 1:
                    cK = max(cK, v - s)
                elif s < v:
                    ok = False
                    break
            if ok:
                t = sum(combo) + cK
                if best is None or t < best[0]:
                    best = (t,) + combo + (cK,)
        CW[:, b] = best[1:]

    # per-dst greedy: assign each category's edges to windows, filling the
    # outermost windows first; verify and bump caps on any overflow.
    dblk = (rank_of // NCORES) // BLK            # block of each node as dst
    capd = CW[:, dblk]                           # [K, N]
    loads = np.zeros((K, N), np.int64)

    def worder(i, j):
        mid = (K - 1) / 2.0
        return sorted(range(i, j + 1), key=lambda w: (-abs(w - mid), w))

    quota = {}
    for (i, j) in sorted(cats, key=lambda ij: ij[1] - ij[0]):
        n = ncat[(i, j)].copy()
        q = np.zeros((K, N), np.int64)
        for w in worder(i, j):
            take = np.minimum(n, np.maximum(capd[w] - loads[w], 0))
            q[w] += take
            loads[w] += take
            n = n - take
        if (n > 0).any():
            # all candidate windows at cap: dump into max-slack window
            sub = np.stack([capd[w] - loads[w] for w in range(i, j + 1)])
            wb = np.argmax(sub, axis=0) + i
            for w in range(i, j + 1):
                m = (wb == w) & (n > 0)
                q[w] += np.where(m, n, 0)
                loads[w] += np.where(m, n, 0)
        quota[(i, j)] = q

    # bump caps to the achieved block maxima (greedy may exceed the LP)
    for w in range(K):
        CW[w] = np.maximum(CW[w], blockmax(loads[w]))

    # ---- per-edge placement ----
    es = np.argsort(dst, kind="stable")
    d_sorted = dst[es]
    f_sorted = f[es]
    l_sorted = l[es]
    row_sorted = row[es]

    w_e = np.zeros(E, dtype=np.int64)
    for (i, j) in cats:
        mi = np.where((f_sorted == i) & (l_sorted == j))[0]
        if len(mi) == 0:
            continue
        if i == j:
            w_e[mi] = i
            continue
        r = _cumcount(d_sorted[mi])
        dd = d_sorted[mi]
        q = quota[(i, j)]
        cum = np.zeros(N, np.int64)
        wsel = np.full(len(mi), -1, np.int64)
        for w in worder(i, j):
            nxt = cum + q[w]
            pick = (wsel < 0) & (r >= cum[dd]) & (r < nxt[dd])
            wsel[pick] = w
            cum = nxt
        assert (wsel >= 0).all()
        w_e[mi] = wsel

    slotW = np.zeros(E, dtype=np.int64)
    for w in range(K):
        mi = np.where(w_e == w)[0]
        slotW[mi] = _cumcount(d_sorted[mi])

    dr_ = rank_of[d_sorted]
    k_e = dr_ % NCORES
    j_e = dr_ // NCORES
    b_e = j_e // BLK
    p_e = j_e % BLK

    # idx16 strip layout: per (w, b) a run of CW*8 int16 columns
    n16 = (CW * BLK) // 16                       # [K, NB]
    flat16 = n16.reshape(-1)
    starts16 = np.zeros(K * NB, dtype=np.int64)
    np.cumsum(flat16[:-1], out=starts16[1:])
    off16 = starts16.reshape(K, NB)
    TOT16 = int(flat16.sum())

    idx16 = np.empty((NCORES, 16, TOT16), dtype=np.int16)
    for w in range(K):
        for b in range(NB):
            o = int(off16[w][b])
            idx16[:, :, o:o + int(n16[w][b])] = np.int16(WDUM[w])
    relrow = row_sorted - np.array(WSS)[w_e]
    assert (relrow >= 0).all() and (relrow < WIN).all()
    i_lin = slotW * BLK + p_e
    col = off16[w_e, b_e] + i_lin // 16
    prow = i_lin % 16
    idx16[k_e, prow, col] = relrow.astype(np.int16)

    nodes_kj = np.full((NCORES, NPC), -1, dtype=np.int64)
    r_all = 8 * np.tile(np.arange(NPC), (NCORES, 1)) + np.arange(NCORES)[:, None]
    m = r_all < N
    nodes_kj[m] = order[r_all[m]]
    return dict(CW=CW, off16=off16, n16=n16, TOT16=TOT16,
                idx16=idx16, nodes_kj=nodes_kj)


def _build_program(CW, off16, TOT16):
    import concourse.bass as bass
    import concourse.tile as tile
    from concourse import bacc, mybir
    from concourse.masks import make_identity

    f32 = mybir.dt.float32
    bf16 = mybir.dt.bfloat16
    i32 = mybir.dt.int32
    i16 = mybir.dt.int16
    AF = mybir.ActivationFunctionType
    OP = mybir.AluOpType
    RG = [list(range(NCORES))]

    CT = CW.sum(axis=0)                          # [NB]

    # per-window slot prefix sums: window strips are contiguous across
    # blocks, so gathers are emitted as ring-buffer chunks of exactly 8
    # slots (1024 idx) that freely span block boundaries.
    Ssum = np.zeros((K, NB + 1), np.int64)
    Ssum[:, 1:] = CW.cumsum(axis=1)
    CWsum = [int(Ssum[w, NB]) for w in range(K)]
    RING = 40            # ring slots per window (multiple of 8)
    PRE = 16             # prefetch depth in slots

    nc = bacc.Bacc(None, target_bir_lowering=False)

    # ---- external I/O ----
    emb_idx = nc.declare_dram_parameter("emb_idx", [128, NPC // 16], i16,
                                        isOutput=False)
    edge_i16 = nc.declare_dram_parameter("edge_i16", [128, TOT16], i16,
                                         isOutput=False)
    snorm = nc.declare_dram_parameter("snorm", [BLK, NB], f32, isOutput=False)
    embed = nc.declare_dram_parameter("embed", [200, HD], bf16, isOutput=False)
    Wexts = [nc.declare_dram_parameter(f"Wext{i}", [HD, HD + 8], bf16,
                                       isOutput=False) for i in range(3)]
    gammas = [nc.declare_dram_parameter(f"gamma{i}", [HD, 1], f32,
                                        isOutput=False) for i in range(3)]
    betas = [nc.declare_dram_parameter(f"beta{i}", [HD, 1], f32,
                                       isOutput=False) for i in range(3)]
    cls1_w = nc.declare_dram_parameter("cls1_w", [HD, 64], bf16, isOutput=False)
    cls1_b = nc.declare_dram_parameter("cls1_b", [64, 1], f32, isOutput=False)
    cls2_w = nc.declare_dram_parameter("cls2_w", [64, 2], bf16, isOutput=False)
    cls2_b = nc.declare_dram_parameter("cls2_b", [2, 1], f32, isOutput=False)
    mask48 = nc.declare_dram_parameter("mask48", [BLK, 1], f32, isOutput=False)
    elb48 = nc.declare_dram_parameter("elb48", [BLK, 1], f32, isOutput=False)
    out_logits = nc.declare_dram_parameter("out_logits", [2, NPC], f32,
                                           isOutput=True)

    # ---- internal DRAM ----
    table = nc.dram_tensor("table", [NPAD, RW], f32, addr_space="Shared")
    fe_local = nc.dram_tensor("fe_local", [NPC, RW], f32)
    stats_in = nc.dram_tensor("stats_in", [HD, 2], f32)
    stats_out = nc.dram_tensor("stats_out", [NCORES * HD, 2], f32,
                               addr_space="Shared")

    CCH = 512
    NCH = NPC // CCH
    assert NPC == NCH * CCH + 128

    with tile.TileContext(nc) as tc:
        with (
            tc.tile_pool(name="res", bufs=1) as res,
            tc.tile_pool(name="sb", bufs=2) as sb,
            tc.tile_pool(name="ps", bufs=4, space="PSUM") as ps,
        ):
            gbw = [res.tile([BLK, RING, 256], bf16, name=f"gbw{w}")
                   for w in range(K)]
            hv = res.tile([BLK, NB * HD], bf16)
            hT = res.tile([BLK, NB * HD], bf16)
            scr = res.tile([BLK, NB * HD], bf16)
            erp = res.tile([BLK, NB * H], f32)
            eidx = res.tile([128, TOT16], i16)
            snorm_sb = res.tile([BLK, NB], f32)
            ident = res.tile([BLK, BLK], f32)
            Wsb = [res.tile([HD, HD + 8], bf16, name=f"Wsb{i}") for i in range(3)]
            gam_sb = [res.tile([HD, 1], f32, name=f"gam{i}") for i in range(3)]
            bet_sb = [res.tile([HD, 1], f32, name=f"bet{i}") for i in range(3)]
            c1w = res.tile([HD, 64], bf16)
            c1b = res.tile([64, 1], f32)
            c2w = res.tile([64, 2], bf16)
            c2b = res.tile([2, 1], f32)
            emb_sb = res.tile([128, NPC // 16], i16)
            m48 = res.tile([BLK, 1], f32)
            eb48 = res.tile([BLK, 1], f32)
            ident_bf = res.tile([BLK, BLK], bf16)
            ones_col = res.tile([BLK, 1], bf16)

            make_identity(nc, ident[:])
            nc.vector.tensor_copy(out=ident_bf[:], in_=ident[:])
            nc.vector.memset(ones_col[:], 1.0)
            # emb_sb first: it gates the embed gather -> layer-0 head chain;
            # the 2MB edge-index strip is not needed until phase B.
            nc.sync.dma_start(out=emb_sb[:], in_=emb_idx[:])
            nc.sync.dma_start(out=snorm_sb[:], in_=snorm[:])
            for i in range(3):
                nc.sync.dma_start(out=Wsb[i][:], in_=Wexts[i][:])
                nc.sync.dma_start(out=gam_sb[i][:], in_=gammas[i][:])
                nc.sync.dma_start(out=bet_sb[i][:], in_=betas[i][:])
            nc.sync.dma_start(out=c1w[:], in_=cls1_w[:])
            nc.sync.dma_start(out=c1b[:], in_=cls1_b[:])
            nc.sync.dma_start(out=c2w[:], in_=cls2_w[:])
            nc.sync.dma_start(out=c2b[:], in_=cls2_b[:])
            nc.sync.dma_start(out=m48[:], in_=mask48[:])
            nc.sync.dma_start(out=eb48[:], in_=elb48[:])
            nc.sync.dma_start(out=eidx[:], in_=edge_i16[:])

            # ---- embed gather via dma_gather (slot j -> lane j%128, block
            # j//128), chunks of <=1024 indices ----
            j0 = 0
            while j0 < NPC:
                n = min(NMAX, NPC - j0)
                nc.gpsimd.dma_gather(
                    hv[:, (j0 // BLK) * HD:((j0 + n) // BLK) * HD].rearrange(
                        "p (b f) -> p b f", b=n // BLK),
                    embed[:],
                    emb_sb[:, j0 // 16:(j0 + n) // 16],
                    n, n, HD, elem_step=HD)
                j0 += n
            nc.vector.tensor_scalar_mul(out=hv[:, 48 * HD:], in0=hv[:, 48 * HD:],
                                        scalar1=m48[:])

            def elu_inplace(dst_ap, scr_ap, ts=None):
                """dst = elu(dst). 1 Act + 3 vector ops; `ts` picks the engine
                for the linear-path op (Pool when it is otherwise idle)."""
                ts = ts or nc.vector
                nc.scalar.activation(out=scr_ap, in_=dst_ap, func=AF.Exp)
                nc.vector.tensor_scalar_min(out=scr_ap, in0=scr_ap, scalar1=1.0)
                ts.tensor_scalar(out=dst_ap, in0=dst_ap, scalar1=0.0,
                                 scalar2=-1.0, op0=OP.max, op1=OP.add)
                nc.vector.tensor_add(out=dst_ap, in0=dst_ap, in1=scr_ap)

            tab_bf = table[:].bitcast(bf16)          # [NPAD, 256]

            for li in range(3):
                residual = li > 0
                # ---- phase A: per-block BN+ELU + transpose (pass 1) and
                # feat/el/er matmul + table rows (pass 2), interleaved at a
                # 2-block lag so in-order engine queues pipeline both. ----
                def phase_a1(b):
                    hTb = hT[:, b * HD:(b + 1) * HD]
                    if li == 0:
                        pt = ps.tile([BLK, BLK], bf16, space="PSUM", name="ptb",
                                     tag="pt", bufs=2)
                        nc.tensor.transpose(out=pt[:],
                                            in_=hv[:, b * HD:(b + 1) * HD],
                                            identity=ident_bf[:])
                        nc.vector.tensor_copy(out=hTb, in_=pt[:])
                        return
                    # hT_b = elu(bn(hT_b)): exp path fused with BN on Act
                    # (exp(sc*x+bs)); linear path on the otherwise-idle Pool
                    # engine; combine on DVE.
                    scrb = scr[:, b * HD:(b + 1) * HD]
                    nc.scalar.activation(out=scrb, in_=hTb, func=AF.Exp,
                                         bias=bs[:], scale=sc[:])
                    nc.gpsimd.tensor_scalar(out=hTb, in0=hTb, scalar1=sc[:],
                                            scalar2=bs[:], op0=OP.mult,
                                            op1=OP.add)
                    nc.gpsimd.tensor_scalar(out=hTb, in0=hTb, scalar1=0.0,
                                            scalar2=-1.0, op0=OP.max,
                                            op1=OP.add)
                    nc.vector.tensor_scalar_min(out=scrb, in0=scrb, scalar1=1.0)
                    nc.vector.tensor_add(out=hTb, in0=hTb, in1=scrb)
                    pt = ps.tile([BLK, BLK], bf16, space="PSUM", name="ptb",
                                 tag="pt", bufs=2)
                    nc.tensor.transpose(out=pt[:], in_=hTb,
                                        identity=ident_bf[:])
                    if b == NB - 1:
                        nc.vector.tensor_scalar_mul(
                            out=hv[:, b * HD:(b + 1) * HD], in0=pt[:],
                            scalar1=m48[:])
                    else:
                        nc.vector.tensor_copy(
                            out=hv[:, b * HD:(b + 1) * HD], in_=pt[:])

                FG = 7           # fe rows batched per table-row DMA
                feg = [None]

                def phase_a2(b):
                    hTb = hT[:, b * HD:(b + 1) * HD]
                    pf = ps.tile([BLK, HD + 8], f32, space="PSUM", name="pf",
                                 tag="mm", bufs=2)
                    nc.tensor.matmul(out=pf[:], lhsT=hTb,
                                     rhs=Wsb[li][:], start=True, stop=True)
                    gi = b % FG
                    if gi == 0:
                        feg[0] = sb.tile([BLK, FG, 68], f32, tag="fe", bufs=3,
                                         name="feg")
                    fe = feg[0]
                    nc.scalar.activation(out=fe[:, gi, 0:64].bitcast(bf16),
                                         in_=pf[:, 0:HD], func=AF.Identity)
                    nc.vector.tensor_copy(out=fe[:, gi, 64:68],
                                          in_=pf[:, HD:HD + 4])
                    nc.vector.tensor_copy(out=erp[:, b * H:(b + 1) * H],
                                          in_=pf[:, HD + 4:HD + 8])
                    if b == NB - 1:
                        nc.vector.tensor_scalar(
                            out=fe[:, gi, 64:68], in0=fe[:, gi, 64:68],
                            scalar1=m48[:], scalar2=eb48[:],
                            op0=OP.mult, op1=OP.add)
                    if gi == FG - 1:
                        b0 = b - FG + 1
                        nc.sync.dma_start(
                            out=fe_local[b0 * BLK:(b + 1) * BLK, 0:68]
                            .rearrange("(g p) w -> p g w", p=BLK),
                            in_=fe[:])

                LAG = 4
                for b in range(NB + LAG):
                    if b < NB:
                        phase_a1(b)
                    if b >= LAG:
                        phase_a2(b - LAG)
                nc.gpsimd.collective_compute(
                    "AllGather", OP.bypass, replica_groups=RG,
                    ins=[fe_local[:]], outs=[table[:]],
                )
                # ---- phase B+C fused per block ----
                pst0 = ps.tile([HD, 1], f32, space="PSUM", name="pst0",
                               tag="pst0", bufs=1)
                pst1 = ps.tile([HD, 1], f32, space="PSUM", name="pst1",
                               tag="pst1", bufs=1)
                issued = [0] * K

                def ensure(w, upto):
                    upto = min(upto, CWsum[w])
                    base = int(off16[w][0])
                    while issued[w] * 8 < upto:
                        s0 = issued[w] * 8
                        cl = min(NMAX // BLK, CWsum[w] - s0)
                        rp = s0 % RING
                        nc.gpsimd.dma_gather(
                            gbw[w][:, rp:rp + cl],
                            tab_bf[WSS[w]:WSS[w] + WIN],
                            eidx[:, base + s0 * 8:base + s0 * 8 + cl * 8],
                            cl * BLK, cl * BLK, 256, elem_step=256)
                        issued[w] += 1

                pend = {}

                def bc_B(b):
                    # Only cover this block's own slots before its reads are
                    # emitted: a prefetched gather that rewrites a ring lap
                    # whose readers are not yet issued would be ordered AHEAD
                    # of those reads (write-before-read on the ring region).
                    # The deeper prefetch happens after the block's ops.
                    for w in range(K):
                        ensure(w, int(Ssum[w][b + 1]))
                    Cb = int(CT[b])
                    parts = []       # (window, ring_pos, len, eb_offset)
                    coff = 0
                    for w in range(K):
                        cw = int(CW[w][b])
                        if cw == 0:
                            continue
                        s = int(Ssum[w][b]) % RING
                        if s + cw <= RING:
                            parts.append((w, s, cw, coff))
                        else:
                            parts.append((w, s, RING - s, coff))
                            parts.append((w, 0, cw - (RING - s),
                                          coff + (RING - s)))
                        coff += cw
                    eb = sb.tile([BLK, Cb, H], f32, tag="eb", bufs=6)
                    for w, rp, cw, co in parts:
                        er_ap = erp[:, b * H:(b + 1) * H].unsqueeze(1) \
                            .to_broadcast((BLK, cw, H))
                        nc.vector.tensor_tensor(
                            out=eb[:, co:co + cw],
                            in0=gbw[w][:, rp:rp + cw, 128:136].bitcast(f32),
                            in1=er_ap, op=OP.add)
                    if _INTERP_DEBUG:
                        lb = sb.tile([BLK, Cb, H], f32, tag="lb", bufs=3)
                        nc.vector.tensor_scalar_mul(out=lb[:], in0=eb[:],
                                                    scalar1=NEG)
                        nc.vector.tensor_tensor(out=eb[:], in0=eb[:],
                                                in1=lb[:], op=OP.max)
                    else:
                        nc.scalar.activation(out=eb[:], in_=eb[:],
                                             func=AF.Prelu, alpha=NEG)
                    ab = sb.tile([BLK, Cb, H], bf16, tag="ab", bufs=6)
                    nc.scalar.activation(out=ab[:], in_=eb[:], func=AF.Exp)
                    s4 = sb.tile([BLK, H], f32, tag="s4", bufs=6)
                    nc.vector.tensor_reduce(
                        out=s4[:], in_=ab[:].rearrange("p c h -> p h c"),
                        axis=mybir.AxisListType.X, op=OP.add)
                    nc.vector.tensor_scalar_add(out=s4[:], in0=s4[:],
                                                scalar1=1e-30)
                    rs = sb.tile([BLK, H], f32, tag="rs", bufs=6)
                    nc.vector.reciprocal(out=rs[:], in_=s4[:])
                    if li == 0:
                        nc.vector.tensor_scalar_mul(out=rs[:], in0=rs[:],
                                                    scalar1=snorm_sb[:, b:b + 1])
                    # in-place message scaling: gb_feat *= ab (broadcast over d)
                    for w, rp, cw, co in parts:
                        gfeat = gbw[w][:, rp:rp + cw, 0:128].rearrange(
                            "p c (d h) -> p c d h", d=D)
                        nc.vector.tensor_tensor(
                            out=gfeat, in0=gfeat,
                            in1=ab[:, co:co + cw].unsqueeze(2).to_broadcast(
                                (BLK, cw, D, H)),
                            op=OP.mult)
                    pr = ps.tile([BLK, HD], f32, space="PSUM", name="pr",
                                 tag="mm", bufs=2)
                    ci = 0
                    for w, rp, cw, co in parts:
                        for c in range(cw):
                            nc.tensor.matmul(
                                out=pr[:],
                                lhsT=ident_bf[:],
                                rhs=gbw[w][:, rp + c, 0:128],
                                start=(ci == 0), stop=(ci == Cb - 1))
                            ci += 1
                    pend[b] = (pr, rs)
                    # deep prefetch: safe now that this block's reads exist
                    for w in range(K):
                        ensure(w, int(Ssum[w][b + 1]) + PRE)

                def bc_C(b):
                    pr, rs = pend.pop(b)
                    rstb = sb.tile([BLK, HD], bf16, tag="rstb", bufs=4)
                    nc.vector.tensor_tensor(
                        out=rstb[:].rearrange("p (d h) -> p d h", d=D),
                        in0=pr[:].rearrange("p (d h) -> p d h", d=D),
                        in1=rs[:].unsqueeze(1).to_broadcast((BLK, D, H)),
                        op=OP.mult)
                    scrb = scr[:, b * HD:(b + 1) * HD]
                    if residual:
                        nc.vector.tensor_add(out=rstb[:], in0=rstb[:],
                                             in1=hv[:, b * HD:(b + 1) * HD])
                        elu_inplace(rstb[:], scrb)
                        nc.vector.tensor_scalar_mul(
                            out=rstb[:], in0=rstb[:],
                            scalar1=snorm_sb[:, b:b + 1])
                    nc.scalar.activation(out=scrb, in_=rstb[:], func=AF.Square)
                    nc.tensor.matmul(out=pst0[:], lhsT=rstb[:], rhs=ones_col[:],
                                     start=(b == 0), stop=(b == NB - 1))
                    nc.tensor.matmul(out=pst1[:], lhsT=scrb, rhs=ones_col[:],
                                     start=(b == 0), stop=(b == NB - 1))
                    pt = ps.tile([BLK, BLK], bf16, space="PSUM", name="ptb",
                                 tag="pt", bufs=2)
                    nc.tensor.transpose(out=pt[:], in_=rstb[:],
                                        identity=ident_bf[:])
                    nc.scalar.activation(out=hT[:, b * HD:(b + 1) * HD],
                                         in_=pt[:], func=AF.Identity)

                for b in range(NB + 1):
                    if b < NB:
                        bc_B(b)
                    if b >= 1:
                        bc_C(b - 1)
                st = sb.tile([HD, 2], f32, tag="st", bufs=1)
                nc.vector.tensor_copy(out=st[:, 0:1], in_=pst0[:])
                nc.vector.tensor_copy(out=st[:, 1:2], in_=pst1[:])
                nc.sync.dma_start(out=stats_in[:], in_=st[:])
                nc.gpsimd.collective_compute(
                    "AllGather", OP.bypass, replica_groups=RG,
                    ins=[stats_in[:]], outs=[stats_out[:]],
                )
                st8 = sb.tile([HD, NCORES, 2], f32, tag="st8", bufs=1)
                nc.sync.dma_start(
                    out=st8[:],
                    in_=stats_out[:].rearrange("(k c) s -> c k s", k=NCORES))
                st2 = sb.tile([HD, 2], f32, tag="st2", bufs=1)
                nc.vector.tensor_reduce(
                    out=st2[:], in_=st8[:].rearrange("c k s -> c s k"),
                    axis=mybir.AxisListType.X, op=OP.add)
                mu = sb.tile([HD, 1], f32, tag="mu", bufs=1)
                var = sb.tile([HD, 1], f32, tag="var", bufs=1)
                sc = sb.tile([HD, 1], f32, tag="sc", bufs=1)
                bs = sb.tile([HD, 1], f32, tag="bs", bufs=1)
                nc.vector.tensor_scalar_mul(out=mu[:], in0=st2[:, 0:1],
                                            scalar1=1.0 / N)
                nc.vector.tensor_scalar_mul(out=var[:], in0=st2[:, 1:2],
                                            scalar1=1.0 / N)
                nc.vector.tensor_tensor(out=sc[:], in0=mu[:], in1=mu[:],
                                        op=OP.mult)
                nc.vector.tensor_tensor(out=var[:], in0=var[:], in1=sc[:],
                                        op=OP.subtract)
                nc.vector.tensor_scalar_add(out=var[:], in0=var[:],
                                            scalar1=BN_EPS)
                nc.scalar.activation(out=var[:], in_=var[:], func=AF.Sqrt)
                nc.vector.reciprocal(out=var[:], in_=var[:])
                nc.vector.tensor_tensor(out=sc[:], in0=gam_sb[li][:], in1=var[:],
                                        op=OP.mult)
                nc.vector.tensor_tensor(out=bs[:], in0=mu[:], in1=sc[:],
                                        op=OP.mult)
                nc.vector.tensor_tensor(out=bs[:], in0=bet_sb[li][:], in1=bs[:],
                                        op=OP.subtract)

            # ---- classifier: final BN+ELU fused per chunk, interleaved at a
            # 2-chunk lag so the per-chunk chains pipeline across engines ----
            chunks = [(i * CCH, CCH) for i in range(NCH)] + [(NCH * CCH, 128)]

            def cls_1(c0, cl):
                nc.gpsimd.tensor_scalar(out=hT[:, c0:c0 + cl],
                                        in0=hT[:, c0:c0 + cl], scalar1=sc[:],
                                        scalar2=bs[:], op0=OP.mult, op1=OP.add)
                elu_inplace(hT[:, c0:c0 + cl], scr[:, c0:c0 + cl],
                            ts=nc.gpsimd)

            def cls_2(c0, cl):
                pz = ps.tile([64, CCH], f32, space="PSUM", name="pz",
                             tag="pz", bufs=1)
                nc.tensor.matmul(out=pz[:, 0:cl], lhsT=c1w[:],
                                 rhs=hT[:, c0:c0 + cl], start=True, stop=True)
                zc = sb.tile([64, CCH], bf16, tag="zc", bufs=2)
                nc.scalar.activation(out=zc[:, 0:cl], in_=pz[:, 0:cl],
                                     func=AF.Relu, bias=c1b[:])
                pl = ps.tile([2, CCH], f32, space="PSUM", name="pl",
                             tag="pl", bufs=1)
                nc.tensor.matmul(out=pl[:, 0:cl], lhsT=c2w[:], rhs=zc[:, 0:cl],
                                 start=True, stop=True)
                lg = sb.tile([2, CCH], f32, tag="lg", bufs=2)
                nc.scalar.activation(out=lg[:, 0:cl], in_=pl[:, 0:cl],
                                     func=AF.Identity, bias=c2b[:])
                nc.sync.dma_start(out=out_logits[:, c0:c0 + cl], in_=lg[:, 0:cl])

            for i in range(len(chunks) + 2):
                if i < len(chunks):
                    cls_1(*chunks[i])
                if i >= 2:
                    cls_2(*chunks[i - 2])
    nc.compile()
    return nc


_PLAN_CACHE = {}


def kernel(**inputs):
    from concourse.bass_utils import run_bass_kernel_spmd

    inp = {k: np.asarray(v) for k, v in inputs.items()}
    import hashlib
    fp = hashlib.sha1()
    fp.update(np.ascontiguousarray(inp["src"]).tobytes())
    fp.update(np.ascontiguousarray(inp["dst"]).tobytes())
    fp = fp.hexdigest()
    if fp in _PLAN_CACHE:
        plan = _PLAN_CACHE[fp]
    else:
        plan = _build_plan(inp["src"], inp["dst"])
        _PLAN_CACHE[fp] = plan
    CW = plan["CW"]
    off16, TOT16 = plan["off16"], plan["TOT16"]
    nodes_kj = plan["nodes_kj"]

    key = (TOT16, tuple(CW.reshape(-1).tolist()))
    if key not in _CACHE:
        _CACHE[key] = _build_program(CW, off16, TOT16)
    nc = _CACHE[key]

    # host-side weight prep. Feature channels are stored d-major on device:
    # new col j = d*H + h holds original channel h*D + d.
    jn = np.arange(HD)
    PERM = (jn % H) * D + (jn // H)          # PERM[new] = old

    def _to_bf16(x):
        import ml_dtypes
        return np.asarray(x, np.float32).astype(ml_dtypes.bfloat16)

    def wext(W, al, ar, perm_rows):
        albd = np.zeros((HD, H), np.float32)
        arbd = np.zeros((HD, H), np.float32)
        for hh in range(H):
            albd[hh * D:(hh + 1) * D, hh] = al[hh]
            arbd[hh * D:(hh + 1) * D, hh] = ar[hh]
        full = np.concatenate(
            [W[:, PERM], W @ albd, W @ arbd], axis=1).astype(np.float32)
        if perm_rows:
            full = full[PERM, :]
        return _to_bf16(full)

    m48v = np.ones((BLK, 1), np.float32); m48v[106:] = 0.0
    e48v = np.zeros((BLK, 1), np.float32); e48v[106:] = EL_DUMMY
    common = {
        "mask48": m48v, "elb48": e48v,
        "embed": _to_bf16(inp["embed"]),
        "cls1_w": _to_bf16(inp["cls1_w"].astype(np.float32)[PERM, :]),
        "cls1_b": inp["cls1_b"].reshape(64, 1).astype(np.float32),
        "cls2_w": _to_bf16(inp["cls2_w"]),
        "cls2_b": inp["cls2_b"].reshape(2, 1).astype(np.float32),
    }
    for i in range(3):
        common[f"Wext{i}"] = wext(inp[f"W{i}"], inp[f"al{i}"], inp[f"ar{i}"],
                                  perm_rows=(i > 0))
        common[f"gamma{i}"] = inp[f"gamma{i}"].astype(np.float32)[PERM] \
            .reshape(HD, 1)
        common[f"beta{i}"] = inp[f"beta{i}"].astype(np.float32)[PERM] \
            .reshape(HD, 1)

    x = inp["x"].astype(np.int64)
    sn = inp["snorm_n"].reshape(-1).astype(np.float32)
    in_maps = []
    for k in range(NCORES):
        nk = nodes_kj[k]
        m = nk >= 0
        ei = np.zeros(NPC, dtype=np.int32)
        ei[m] = x[nk[m]].astype(np.int32)
        snk = np.zeros(NPC, dtype=np.float32)
        snk[m] = sn[nk[m]]
        im = dict(common)
        im["emb_idx"] = np.tile(
            ei.astype(np.int16).reshape(NPC // 16, 16).T, (8, 1))
        im["snorm"] = snk.reshape(NB, BLK).T.copy()
        im["edge_i16"] = np.tile(plan["idx16"][k], (8, 1))  # [128, TOT16]
        in_maps.append(im)

    res = run_bass_kernel_spmd(nc, in_maps, list(range(NCORES))).results

    out = np.zeros((N, 2), np.float32)
    for k in range(NCORES):
        lg = res[k]["out_logits"].T
        nk = nodes_kj[k]
        m = nk >= 0
        out[nk[m]] = lg[m]
    return out
